# revision 22
# baseline (speedup 1.0000x reference)
"""GAT layer (LayerNorm -> GATConv(heads=1) -> residual ReLU) on 8 trn2 NeuronCores.

Sharding: destination-node parallel. Each core owns N/8 contiguous nodes,
computes the node transform for its shard, AllGathers the transformed table,
then processes the edges whose destination lands in its shard.

Key design points (v2, rebuilt after profiling the 768B-row baseline):
- The node table is fp16, 256 B/row (the dma_gather minimum): a host-side
  orthonormal rotation Q puts att_src along coordinate 127, so the gathered
  row IS [rotated feats | a_src] with zero extra columns; the rotation is
  undone after the softmax-weighted scatter by one 128x128 matmul per dst
  block (Q is orthogonal, applied to the accumulated sums).
- No per-edge a_dst gather: a_dst per edge = ohT_cb^T @ adst_block via a
  1-column matmul per 128-edge column block, with one-hot tables streamed
  from the host as fp8 (exact 0/1).
- No DVE one-hot builds: the scatter matmul uses lhsT = host fp8 one-hot,
  rhs = gathered rows * ee (folded on ACT/DVE), with a constant ones column
  in the rhs producing the softmax denominator in the same matmul.
- Feature gathers round-robin over 4 SWDGE queues to overlap DMA drains.
"""

import numpy as np
import ml_dtypes

import concourse.bacc as bacc
import concourse.mybir as mybir
import concourse.tile as tile
from concourse.bass_utils import run_bass_kernel_spmd

F32 = mybir.dt.float32
F16 = mybir.dt.float16
F8 = mybir.dt.float8e4
I16 = mybir.dt.int16
AX = mybir.AxisListType
OP = mybir.AluOpType
AF = mybir.ActivationFunctionType

N = 50000
D = 128
E = 600000
NCORES = 8
SHARD = N // NCORES            # 6250
NBLK = (SHARD + 127) // 128    # 49 dst blocks per core
PAD_SHARD = NBLK * 128         # 6272
LAST_ROWS = SHARD - (NBLK - 1) * 128  # 106
HALF = 32768                   # int16 index split point for the global table
NEG_SLOPE = 0.2
LN_EPS = 1e-5
GBLK = 4                       # dst blocks per gather group
NQ = 4                         # SWDGE queues for gathers


def _build_program(tlo, thi, na1):
    """One SPMD program; per-core behaviour differs only through its inputs."""
    nc = bacc.Bacc("TRN2", num_devices=NCORES, debug=False, num_swdge_queues=NQ)

    CB = sum(tlo) + sum(thi)   # total column-blocks (tiles) per core

    x_shard = nc.dram_tensor("x_shard", [PAD_SHARD, D], F32, kind="ExternalInput")
    wextq = nc.dram_tensor("wextq", [D, 129], F16, kind="ExternalInput")
    c2q = nc.dram_tensor("c2q", [128, 129], F32, kind="ExternalInput")
    ident = nc.dram_tensor("ident", [128, 128], F32, kind="ExternalInput")
    qmat = nc.dram_tensor("qmat", [128, 128], F16, kind="ExternalInput")
    feat_idx = nc.dram_tensor("feat_idx", [128, CB * 8], I16, kind="ExternalInput")
    oh_d = nc.dram_tensor("oh_d", [128, CB * 128], F8, kind="ExternalInput")
    ohT_d = nc.dram_tensor("ohT_d", [128, CB * 128], F8, kind="ExternalInput")
    out_shard = nc.dram_tensor("out_shard", [SHARD, D], F32, kind="ExternalOutput")

    # group structure (static, identical on every core)
    groups = []
    cb0 = 0
    for g0 in range(0, NBLK, GBLK):
        blocks = list(range(g0, min(NBLK, g0 + GBLK)))
        nlo = sum(tlo[b] for b in blocks)
        nhi = sum(thi[b] for b in blocks)
        groups.append((blocks, cb0, nlo, nhi))
        cb0 += nlo + nhi
    assert cb0 == CB
    CBG_MAX = max(nlo + nhi for _, _, nlo, nhi in groups)

    # cb -> owning block (within its group), same order the host uses
    cb_block = [0] * CB
    for blocks, cb0g, nlo, nhi in groups:
        off = cb0g
        for b in blocks:
            for _ in range(tlo[b]):
                cb_block[off] = b
                off += 1
        for b in blocks:
            for _ in range(thi[b]):
                cb_block[off] = b
                off += 1

    with tile.TileContext(nc) as tc:
        with (
            tc.tile_pool(name="dram", bufs=1, space="DRAM") as dram,
            tc.tile_pool(name="consts", bufs=1) as cpool,
            tc.tile_pool(name="xres", bufs=1) as xpool,
            tc.tile_pool(name="xresh", bufs=1) as xhpool,
        ):
            xb_shard = dram.tile([SHARD, D], F16)
            xb_full = dram.tile([N, D], F16, addr_space="Shared")

            ident_sb = cpool.tile([128, 128], F32)
            nc.sync.dma_start(ident_sb[:], ident[:, :])
            q_sb = cpool.tile([128, 128], F16)
            nc.sync.dma_start(q_sb[:], qmat[:, :])
            wext_sb = cpool.tile([D, 129], F16)
            nc.sync.dma_start(wext_sb[:], wextq[:, :])
            identh_sb = cpool.tile([128, 128], F16)
            nc.scalar.copy(identh_sb[:], ident_sb[:])
            c2_sb = cpool.tile([128, 129], F32)
            nc.sync.dma_start(c2_sb[:], c2q[:, :])
            eps_sb = cpool.tile([128, 1], F32)
            nc.vector.memset(eps_sb[:], LN_EPS)
            fidx_sb = cpool.tile([128, CB * 8], I16)
            nc.sync.dma_start(fidx_sb[:], feat_idx[:, :])
            adst_sb = cpool.tile([128, NBLK], F16)

            x_tiles = []
            for i in range(NBLK):
                xt = xpool.tile([128, D], F32, tag=f"xres{i}")
                nc.sync.dma_start(xt[:], x_shard[i * 128 : (i + 1) * 128, :])
                x_tiles.append(xt)

            xh_tiles = []
            # ---------------- Phase A: node transform on own shard ---------
            with (
                tc.tile_pool(name="a_small", bufs=8) as spool,
                tc.tile_pool(name="a_xnp", bufs=3) as xnppool,
                tc.tile_pool(name="a_xnpT", bufs=3) as xnptpool,
                tc.tile_pool(name="a_xpe", bufs=3) as xpepool,
                tc.tile_pool(name="a_tb", bufs=3) as tbpool,
                tc.tile_pool(name="a_ps_t", bufs=2, space="PSUM") as psa,
                tc.tile_pool(name="a_ps_m", bufs=2, space="PSUM") as psb,
            ):
                for i in range(NBLK):
                    xt = x_tiles[i]
                    rows = 128 if i < NBLK - 1 else LAST_ROWS
                    stats = spool.tile([128, 6], F32, tag="stats")
                    nc.vector.bn_stats(stats[:], xt[:])
                    mv = spool.tile([128, 2], F32, tag="mv")
                    nc.vector.bn_aggr(mv[:], stats[:])
                    std = spool.tile([128, 1], F32, tag="std")
                    nc.scalar.activation(
                        std[:], mv[:, 1:2], AF.Sqrt, bias=eps_sb[:, 0:1]
                    )
                    rstd = spool.tile([128, 1], F32, tag="rstd")
                    nc.vector.reciprocal(rstd[:], std[:])
                    xnp = xnppool.tile([128, D], F16)
                    nc.vector.tensor_scalar(
                        xnp[:], xt[:], mv[:, 0:1], rstd[:, 0:1], OP.subtract, OP.mult
                    )
                    xh = xhpool.tile([128, D], F16, tag=f"xh{i}")
                    nc.scalar.copy(xh[:], xt[:])
                    xh_tiles.append(xh)
                    pt = psa.tile([128, 128], F16, space="PSUM")
                    nc.tensor.transpose(pt[:], xnp[:], identh_sb[:])
                    xnpT = xnptpool.tile([128, 128], F16)
                    nc.scalar.copy(xnpT[:], pt[:])
                    pm = psb.tile([128, 129], F32, space="PSUM")
                    nc.tensor.matmul(
                        pm[:], lhsT=xnpT[:], rhs=wext_sb[:], start=True, stop=True
                    )
                    xpe = xpepool.tile([128, 129], F32)
                    nc.vector.tensor_tensor(xpe[:], pm[:], c2_sb[:], OP.add)
                    tb = tbpool.tile([128, D], F16, tag="tb")
                    nc.scalar.copy(tb[:], xpe[:, 0:128])
                    nc.sync.dma_start(
                        xb_shard[i * 128 : i * 128 + rows, :], tb[:rows, :]
                    )
                    nc.scalar.copy(adst_sb[:, i : i + 1], xpe[:, 128:129])

            nc.gpsimd.collective_compute(
                "AllGather",
                OP.bypass,
                replica_groups=[list(range(NCORES))],
                ins=[xb_shard[:, :]],
                outs=[xb_full[:, :]],
            )

            # ---------------- Phase B: edge aggregation --------------------
            with (
                tc.tile_pool(name="b_g", bufs=4) as gpool,
                tc.tile_pool(name="b_oh", bufs=2) as ohpool,
                tc.tile_pool(name="b_ohT", bufs=2) as ohtpool,
                tc.tile_pool(name="b_e", bufs=2) as epool,
                tc.tile_pool(name="b_gfe", bufs=2) as gfepool,
                tc.tile_pool(name="b_blk", bufs=4) as blkpool,
                tc.tile_pool(name="b_ps_a", bufs=2, space="PSUM") as ps_adst,
                tc.tile_pool(name="b_ps_s", bufs=2, space="PSUM") as ps_sc,
                tc.tile_pool(name="b_ps_t", bufs=2, space="PSUM") as ps_tp,
                tc.tile_pool(name="b_ps_o", bufs=2, space="PSUM") as ps_out,
            ):
                qctr = 0
                for gi, (blocks, cb0, nlo, nhi) in enumerate(groups):
                    cbg = nlo + nhi
                    gf = gpool.tile([128, CBG_MAX, 128], F16, tag="gf")
                    # split each half-table gather in two on separate SWDGE
                    # queues so their DMA drains overlap
                    segs = []
                    if nlo:
                        h1 = (nlo + 1) // 2
                        segs += [(0, h1, 0), (h1, nlo, 0)] if nlo > 1 else [(0, nlo, 0)]
                    if nhi:
                        h2 = (nhi + 1) // 2
                        segs += (
                            [(nlo, nlo + h2, 1), (nlo + h2, cbg, 1)]
                            if nhi > 1
                            else [(nlo, cbg, 1)]
                        )
                    for s0, s1, hf in segs:
                        nc.gpsimd.dma_gather(
                            out_ap=gf[:, s0:s1, :],
                            in_ap=xb_full[0:HALF, :] if hf == 0 else xb_full[HALF:N, :],
                            idxs_ap=fidx_sb[:, (cb0 + s0) * 8 : (cb0 + s1) * 8],
                            num_idxs=(s1 - s0) * 128,
                            num_idxs_reg=(s1 - s0) * 128,
                            elem_size=128,
                            single_packet=False,
                            queue_num=qctr % NQ,
                        )
                        qctr += 1
                    oh = ohpool.tile([128, CBG_MAX, 128], F8, tag="oh")
                    nc.sync.dma_start(
                        oh.rearrange("p a b -> p (a b)")[:, 0 : cbg * 128],
                        oh_d[:, cb0 * 128 : (cb0 + cbg) * 128],
                    )
                    ohT = ohtpool.tile([128, CBG_MAX, 128], F8, tag="ohT")
                    nc.sync.dma_start(
                        ohT.rearrange("p a b -> p (a b)")[:, 0 : cbg * 128],
                        ohT_d[:, cb0 * 128 : (cb0 + cbg) * 128],
                    )

                    # per-edge a_dst via transposed one-hot x per-block vector
                    pa = ps_adst.tile([128, CBG_MAX], F32, space="PSUM")
                    for j in range(cbg):
                        nc.tensor.matmul(
                            pa[:, j : j + 1],
                            lhsT=ohT[:, j, :],
                            rhs=adst_sb[:, cb_block[cb0 + j] : cb_block[cb0 + j] + 1],
                            start=True,
                            stop=True,
                            skip_group_check=True,
                        )

                    # ee = exp(leakyrelu(|att_src|*t127 + a_dst'))
                    e1 = epool.tile([128, CBG_MAX], F32, tag="e1")
                    nc.vector.scalar_tensor_tensor(
                        e1[:, 0:cbg],
                        in0=gf[:, 0:cbg, 127],
                        scalar=float(na1),
                        in1=pa[:, 0:cbg],
                        op0=OP.mult,
                        op1=OP.add,
                    )
                    e3 = epool.tile([128, CBG_MAX], F32, tag="e3")
                    nc.vector.scalar_tensor_tensor(
                        e3[:, 0:cbg],
                        in0=e1[:, 0:cbg],
                        scalar=NEG_SLOPE,
                        in1=e1[:, 0:cbg],
                        op0=OP.mult,
                        op1=OP.max,
                    )
                    ee = epool.tile([128, CBG_MAX], F32, tag="ee")
                    nc.scalar.activation(ee[:, 0:cbg], e3[:, 0:cbg], AF.Exp)

                    # rhs for the scatter matmuls: [t*ee (128) | ee | pad],
                    # built in one batched multiply + one strided column copy
                    gfe = gfepool.tile([128, CBG_MAX, 130], F16, tag="gfe")
                    nc.vector.tensor_tensor(
                        gfe[:, 0:cbg, 0:128],
                        gf[:, 0:cbg, :],
                        ee[:, 0:cbg].to_broadcast([128, cbg, 128]),
                        OP.mult,
                    )
                    nc.vector.tensor_copy(gfe[:, 0:cbg, 128], ee[:, 0:cbg])

                    # scatter matmuls per block
                    lo_off = 0
                    hi_off = nlo
                    for b in blocks:
                        rows = 128 if b < NBLK - 1 else LAST_ROWS
                        cbs = list(range(lo_off, lo_off + tlo[b])) + list(
                            range(hi_off, hi_off + thi[b])
                        )
                        lo_off += tlo[b]
                        hi_off += thi[b]
                        ps = ps_sc.tile([128, 129], F32, space="PSUM")
                        for j, cb in enumerate(cbs):
                            nc.tensor.matmul(
                                ps[:, :],
                                lhsT=oh[:, cb, :],
                                rhs=gfe[:, cb, 0:129],
                                start=(j == 0),
                                stop=(j == len(cbs) - 1),
                            )
                        recip = blkpool.tile([128, 1], F32, tag="recip")
                        nc.vector.reciprocal(recip[:], ps[:, 128:129])
                        scaled = blkpool.tile([128, D], F32, tag="scaled")
                        nc.scalar.activation(
                            scaled[:], ps[:, 0:D], AF.Copy, scale=recip[:, 0:1]
                        )
                        ptp = ps_tp.tile([128, 128], F32, space="PSUM")
                        nc.tensor.transpose(ptp[:], scaled[:], ident_sb[:])
                        scaledT = blkpool.tile([128, D], F16, tag="scaledT")
                        nc.scalar.copy(scaledT[:], ptp[:])
                        po = ps_out.tile([128, 128], F32, space="PSUM")
                        nc.tensor.matmul(
                            po[:], lhsT=scaledT[:], rhs=q_sb[:], start=True, stop=False
                        )
                        nc.tensor.matmul(
                            po[:], lhsT=identh_sb[:], rhs=xh_tiles[b][:],
                            start=False, stop=True,
                        )
                        outt = blkpool.tile([128, D], F32, tag="outt")
                        nc.scalar.activation(outt[:], po[:], AF.Relu)
                        nc.sync.dma_start(
                            out_shard[b * 128 : b * 128 + rows, :], outt[:rows, :]
                        )

    nc.compile()
    return nc


def _wrap_idx(idx):
    """int16 index list -> dma_gather SBUF layout [128, len/16]:
    index i lives at partitions {16g + i%16: g in 0..7}, column i//16."""
    L = len(idx)
    assert L % 16 == 0
    w = idx.reshape(L // 16, 16).T.astype(np.int16)      # [16, L/16]
    return np.tile(w, (8, 1))                            # [128, L/16]


def _host_prep(x, edge_index, ln_gamma, ln_beta, W, att_src, att_dst, bias):
    """Fold parameters, build rotation Q, bucket edges. Numpy only."""
    Wt = W.T.astype(np.float64)
    G = ln_gamma.astype(np.float64)[:, None] * Wt          # [D, D]
    crow = ln_beta.astype(np.float64) @ Wt                 # [D]
    a1 = att_src.astype(np.float64)
    a2 = att_dst.astype(np.float64)
    na1 = float(np.linalg.norm(a1))
    v_dst = G @ a2
    c_dst = float(crow @ a2)
    kc = float(bias.astype(np.float64) @ a1)

    # orthonormal Q with row 127 = att_src direction (row 126: att_dst comp,
    # kept only so Q is deterministic/well-conditioned)
    q127 = a1 / na1
    u = a2 - (a2 @ q127) * q127
    nu = np.linalg.norm(u)
    if nu > 1e-12:
        q126 = u / nu
        P = np.eye(D) - np.outer(q127, q127) - np.outer(q126, q126)
        Uq, _, _ = np.linalg.svd(P)
        Q = np.vstack([Uq[:, :126].T, q126[None, :], q127[None, :]])
    else:
        P = np.eye(D) - np.outer(q127, q127)
        Uq, _, _ = np.linalg.svd(P)
        Q = np.vstack([Uq[:, :127].T, q127[None, :]])

    c2feat = crow + bias.astype(np.float64)
    wextq = np.zeros((D, 129), np.float16)
    wextq[:, 0:128] = (G @ Q.T).astype(np.float16)
    wextq[:, 128] = v_dst.astype(np.float16)
    c2 = np.zeros((129,), np.float32)
    c2[0:128] = (c2feat @ Q.T).astype(np.float32)
    c2[128] = c_dst - kc
    c2b = np.broadcast_to(c2, (128, 129)).copy()

    ident = np.eye(128, dtype=np.float32)
    qmat = Q.astype(np.float16)

    # edges + self loops, sorted by (core, block, src-half)
    src = np.concatenate([edge_index[0], np.arange(N, dtype=np.int64)]).astype(np.int64)
    dst = np.concatenate([edge_index[1], np.arange(N, dtype=np.int64)]).astype(np.int64)
    core = dst // SHARD
    local = dst - core * SHARD
    blk = local // 128
    half = (src >= HALF).astype(np.int64)
    key = ((core * NBLK + blk) * 2 + half)
    order = np.argsort(key, kind="stable")
    src, dst, key = src[order], dst[order], key[order]
    counts = np.bincount(key, minlength=NCORES * NBLK * 2).reshape(NCORES, NBLK, 2)
    tiles = -(-counts // 128)                              # ceil
    tlo = tuple(int(t) for t in tiles[:, :, 0].max(axis=0))
    thi = tuple(int(t) for t in tiles[:, :, 1].max(axis=0))
    CB = sum(tlo) + sum(thi)

    feat_idx = np.zeros((NCORES, CB * 128), np.int16)
    oh = np.zeros((NCORES, 128, CB, 128), np.uint8)
    ohT = np.zeros((NCORES, 128, CB, 128), np.uint8)

    starts = np.zeros(NCORES * NBLK * 2 + 1, np.int64)
    starts[1:] = np.cumsum(counts.reshape(-1))

    # cb offset of each (block, half) segment, same for every core
    seg_off = {}
    cb0 = 0
    for g0 in range(0, NBLK, GBLK):
        blocks = list(range(g0, min(NBLK, g0 + GBLK)))
        off = cb0
        for b in blocks:
            seg_off[(b, 0)] = off
            off += tlo[b]
        for b in blocks:
            seg_off[(b, 1)] = off
            off += thi[b]
        cb0 = off
    assert cb0 == CB

    for c in range(NCORES):
        for b in range(NBLK):
            for hf in range(2):
                gi = (c * NBLK + b) * 2 + hf
                s, e = starts[gi], starts[gi + 1]
                n = int(e - s)
                if n == 0:
                    continue
                off = seg_off[(b, hf)]
                k = np.arange(n) + off * 128
                fi = (src[s:e] - hf * HALF).astype(np.int16)
                feat_idx[c, k] = fi
                p = k % 128
                t = k // 128
                r = (dst[s:e] - (c * SHARD + b * 128)).astype(np.int64)
                oh[c, p, t, r] = 1
                ohT[c, r, t, p] = 1

    oh8 = oh.astype(ml_dtypes.float8_e4m3fn).reshape(NCORES, 128, CB * 128)
    ohT8 = ohT.astype(ml_dtypes.float8_e4m3fn).reshape(NCORES, 128, CB * 128)

    in_maps = []
    for c in range(NCORES):
        xs = np.zeros((PAD_SHARD, D), np.float32)
        xs[0:SHARD] = x[c * SHARD : (c + 1) * SHARD]
        in_maps.append(
            {
                "x_shard": xs,
                "wextq": wextq,
                "c2q": c2b,
                "ident": ident,
                "qmat": qmat,
                "feat_idx": _wrap_idx(feat_idx[c]),
                "oh_d": np.ascontiguousarray(oh8[c]),
                "ohT_d": np.ascontiguousarray(ohT8[c]),
            }
        )
    return tlo, thi, na1, in_maps


_PROGRAM_CACHE = {}


def kernel(x, edge_index, edge_attr, h, batch, ln_gamma, ln_beta, W, att_src,
           att_dst, bias):
    x = np.asarray(x, dtype=np.float32)
    edge_index = np.asarray(edge_index)
    h = np.asarray(h)
    ln_gamma = np.asarray(ln_gamma, dtype=np.float32)
    ln_beta = np.asarray(ln_beta, dtype=np.float32)
    W = np.asarray(W, dtype=np.float32)
    att_src = np.asarray(att_src, dtype=np.float32)
    att_dst = np.asarray(att_dst, dtype=np.float32)
    bias = np.asarray(bias, dtype=np.float32)

    tlo, thi, na1, in_maps = _host_prep(
        x, edge_index, ln_gamma, ln_beta, W, att_src, att_dst, bias
    )
    key = (tlo, thi, round(na1, 6))
    if key not in _PROGRAM_CACHE:
        _PROGRAM_CACHE[key] = _build_program(tlo, thi, na1)
    nc = _PROGRAM_CACHE[key]

    res = run_bass_kernel_spmd(nc, in_maps, core_ids=list(range(NCORES)))
    out = np.concatenate([res.results[c]["out_shard"] for c in range(NCORES)], axis=0)
    return out, h


# revision 25
# speedup vs baseline: 1.0537x; 1.0537x over previous
"""GAT layer (LayerNorm -> GATConv(heads=1) -> residual ReLU) on 8 trn2 NeuronCores.

Sharding: destination-node parallel. Each core owns N/8 contiguous nodes,
computes the node transform for its shard, AllGathers the transformed table,
then processes the edges whose destination lands in its shard.

Key design points (v2, rebuilt after profiling the 768B-row baseline):
- The node table is fp16, 256 B/row (the dma_gather minimum): a host-side
  orthonormal rotation Q puts att_src along coordinate 127, so the gathered
  row IS [rotated feats | a_src] with zero extra columns; the rotation is
  undone after the softmax-weighted scatter by one 128x128 matmul per dst
  block (Q is orthogonal, applied to the accumulated sums).
- No per-edge a_dst gather: a_dst per edge = ohT_cb^T @ adst_block via a
  1-column matmul per 128-edge column block, with one-hot tables streamed
  from the host as fp8 (exact 0/1).
- No DVE one-hot builds: the scatter matmul uses lhsT = host fp8 one-hot,
  rhs = gathered rows * ee (folded on ACT/DVE), with a constant ones column
  in the rhs producing the softmax denominator in the same matmul.
- Feature gathers round-robin over 4 SWDGE queues to overlap DMA drains.
"""

import numpy as np
import ml_dtypes

import concourse.bacc as bacc
import concourse.mybir as mybir
import concourse.tile as tile
from concourse.bass_utils import run_bass_kernel_spmd

F32 = mybir.dt.float32
F16 = mybir.dt.float16
F8 = mybir.dt.float8e4
I16 = mybir.dt.int16
AX = mybir.AxisListType
OP = mybir.AluOpType
AF = mybir.ActivationFunctionType

N = 50000
D = 128
E = 600000
NCORES = 8
SHARD = N // NCORES            # 6250
NBLK = (SHARD + 127) // 128    # 49 dst blocks per core
PAD_SHARD = NBLK * 128         # 6272
LAST_ROWS = SHARD - (NBLK - 1) * 128  # 106
HALF = 32768                   # int16 index split point for the global table
NEG_SLOPE = 0.2
LN_EPS = 1e-5
GBLK = 4                       # dst blocks per gather group
NQ = 4                         # SWDGE queues for gathers


def _build_program(tlo, thi, na1):
    """One SPMD program; per-core behaviour differs only through its inputs."""
    nc = bacc.Bacc("TRN2", num_devices=NCORES, debug=False, num_swdge_queues=NQ)

    CB = sum(tlo) + sum(thi)   # total column-blocks (tiles) per core

    x_shard = nc.dram_tensor("x_shard", [PAD_SHARD, D], F32, kind="ExternalInput")
    wextq = nc.dram_tensor("wextq", [D, 129], F16, kind="ExternalInput")
    c2q = nc.dram_tensor("c2q", [128, 129], F32, kind="ExternalInput")
    ident = nc.dram_tensor("ident", [128, 128], F32, kind="ExternalInput")
    qmat = nc.dram_tensor("qmat", [128, 128], F16, kind="ExternalInput")
    feat_idx = nc.dram_tensor("feat_idx", [128, CB * 8], I16, kind="ExternalInput")
    oh_d = nc.dram_tensor("oh_d", [128, CB * 128], F8, kind="ExternalInput")
    ohT_d = nc.dram_tensor("ohT_d", [128, CB * 128], F8, kind="ExternalInput")
    out_shard = nc.dram_tensor("out_shard", [SHARD, D], F32, kind="ExternalOutput")

    # group structure (static, identical on every core)
    groups = []
    cb0 = 0
    for g0 in range(0, NBLK, GBLK):
        blocks = list(range(g0, min(NBLK, g0 + GBLK)))
        nlo = sum(tlo[b] for b in blocks)
        nhi = sum(thi[b] for b in blocks)
        groups.append((blocks, cb0, nlo, nhi))
        cb0 += nlo + nhi
    assert cb0 == CB
    CBG_MAX = max(nlo + nhi for _, _, nlo, nhi in groups)

    # cb -> owning block (within its group), same order the host uses
    cb_block = [0] * CB
    for blocks, cb0g, nlo, nhi in groups:
        off = cb0g
        for b in blocks:
            for _ in range(tlo[b]):
                cb_block[off] = b
                off += 1
        for b in blocks:
            for _ in range(thi[b]):
                cb_block[off] = b
                off += 1

    with tile.TileContext(nc) as tc:
        with (
            tc.tile_pool(name="dram", bufs=1, space="DRAM") as dram,
            tc.tile_pool(name="consts", bufs=1) as cpool,
            tc.tile_pool(name="xres", bufs=1) as xpool,
            tc.tile_pool(name="xresh", bufs=1) as xhpool,
        ):
            xb_shard = dram.tile([SHARD, D], F16)
            xb_full = dram.tile([N, D], F16, addr_space="Shared")

            ident_sb = cpool.tile([128, 128], F32)
            nc.sync.dma_start(ident_sb[:], ident[:, :])
            q_sb = cpool.tile([128, 128], F16)
            nc.sync.dma_start(q_sb[:], qmat[:, :])
            wext_sb = cpool.tile([D, 129], F16)
            nc.sync.dma_start(wext_sb[:], wextq[:, :])
            identh_sb = cpool.tile([128, 128], F16)
            nc.scalar.copy(identh_sb[:], ident_sb[:])
            c2_sb = cpool.tile([128, 129], F32)
            nc.sync.dma_start(c2_sb[:], c2q[:, :])
            eps_sb = cpool.tile([128, 1], F32)
            nc.vector.memset(eps_sb[:], LN_EPS)
            fidx_sb = cpool.tile([128, CB * 8], I16)
            nc.sync.dma_start(fidx_sb[:], feat_idx[:, :])
            adst_sb = cpool.tile([128, NBLK], F16)

            x_tiles = []
            for i in range(NBLK):
                xt = xpool.tile([128, D], F32, tag=f"xres{i}")
                nc.sync.dma_start(xt[:], x_shard[i * 128 : (i + 1) * 128, :])
                x_tiles.append(xt)

            xh_tiles = []
            # ---------------- Phase A: node transform on own shard ---------
            with (
                tc.tile_pool(name="a_small", bufs=8) as spool,
                tc.tile_pool(name="a_xnp", bufs=3) as xnppool,
                tc.tile_pool(name="a_xnpT", bufs=3) as xnptpool,
                tc.tile_pool(name="a_xpe", bufs=3) as xpepool,
                tc.tile_pool(name="a_tb", bufs=3) as tbpool,
                tc.tile_pool(name="a_ps_t", bufs=2, space="PSUM") as psa,
                tc.tile_pool(name="a_ps_m", bufs=2, space="PSUM") as psb,
            ):
                for i in range(NBLK):
                    xt = x_tiles[i]
                    rows = 128 if i < NBLK - 1 else LAST_ROWS
                    stats = spool.tile([128, 6], F32, tag="stats")
                    nc.vector.bn_stats(stats[:], xt[:])
                    mv = spool.tile([128, 2], F32, tag="mv")
                    nc.vector.bn_aggr(mv[:], stats[:])
                    std = spool.tile([128, 1], F32, tag="std")
                    nc.scalar.activation(
                        std[:], mv[:, 1:2], AF.Sqrt, bias=eps_sb[:, 0:1]
                    )
                    rstd = spool.tile([128, 1], F32, tag="rstd")
                    nc.vector.reciprocal(rstd[:], std[:])
                    xnp = xnppool.tile([128, D], F16)
                    nc.vector.tensor_scalar(
                        xnp[:], xt[:], mv[:, 0:1], rstd[:, 0:1], OP.subtract, OP.mult
                    )
                    xh = xhpool.tile([128, D], F16, tag=f"xh{i}")
                    nc.scalar.copy(xh[:], xt[:])
                    xh_tiles.append(xh)
                    pt = psa.tile([128, 128], F16, space="PSUM")
                    nc.tensor.transpose(pt[:], xnp[:], identh_sb[:])
                    xnpT = xnptpool.tile([128, 128], F16)
                    nc.scalar.copy(xnpT[:], pt[:])
                    pm = psb.tile([128, 129], F32, space="PSUM")
                    nc.tensor.matmul(
                        pm[:], lhsT=xnpT[:], rhs=wext_sb[:], start=True, stop=True
                    )
                    xpe = xpepool.tile([128, 129], F32)
                    nc.vector.tensor_tensor(xpe[:], pm[:], c2_sb[:], OP.add)
                    tb = tbpool.tile([128, D], F16, tag="tb")
                    nc.scalar.copy(tb[:], xpe[:, 0:128])
                    nc.sync.dma_start(
                        xb_shard[i * 128 : i * 128 + rows, :], tb[:rows, :]
                    )
                    nc.scalar.copy(adst_sb[:, i : i + 1], xpe[:, 128:129])

            nc.gpsimd.collective_compute(
                "AllGather",
                OP.bypass,
                replica_groups=[list(range(NCORES))],
                ins=[xb_shard[:, :]],
                outs=[xb_full[:, :]],
            )

            # ---------------- Phase B: edge aggregation --------------------
            with (
                tc.tile_pool(name="b_g", bufs=4) as gpool,
                tc.tile_pool(name="b_oh", bufs=3) as ohpool,
                tc.tile_pool(name="b_ohT", bufs=3) as ohtpool,
                tc.tile_pool(name="b_e", bufs=2) as epool,
                tc.tile_pool(name="b_gfe", bufs=2) as gfepool,
                tc.tile_pool(name="b_blk", bufs=4) as blkpool,
                tc.tile_pool(name="b_ps_a", bufs=2, space="PSUM") as ps_adst,
                tc.tile_pool(name="b_ps_s", bufs=3, space="PSUM") as ps_sc,
                tc.tile_pool(name="b_ps_t", bufs=2, space="PSUM") as ps_tp,
            ):
                ps_out = ps_tp
                qctr = 0
                for gi, (blocks, cb0, nlo, nhi) in enumerate(groups):
                    cbg = nlo + nhi
                    gf = gpool.tile([128, CBG_MAX, 128], F16, tag="gf")
                    # split each half-table gather in two on separate SWDGE
                    # queues so their DMA drains overlap
                    segs = []
                    if nlo:
                        h1 = (nlo + 1) // 2
                        segs += [(0, h1, 0), (h1, nlo, 0)] if nlo > 1 else [(0, nlo, 0)]
                    if nhi:
                        h2 = (nhi + 1) // 2
                        segs += (
                            [(nlo, nlo + h2, 1), (nlo + h2, cbg, 1)]
                            if nhi > 1
                            else [(nlo, cbg, 1)]
                        )
                    for s0, s1, hf in segs:
                        nc.gpsimd.dma_gather(
                            out_ap=gf[:, s0:s1, :],
                            in_ap=xb_full[0:HALF, :] if hf == 0 else xb_full[HALF:N, :],
                            idxs_ap=fidx_sb[:, (cb0 + s0) * 8 : (cb0 + s1) * 8],
                            num_idxs=(s1 - s0) * 128,
                            num_idxs_reg=(s1 - s0) * 128,
                            elem_size=128,
                            single_packet=False,
                            queue_num=qctr % NQ,
                        )
                        qctr += 1
                    oh = ohpool.tile([128, CBG_MAX, 128], F8, tag="oh")
                    nc.sync.dma_start(
                        oh.rearrange("p a b -> p (a b)")[:, 0 : cbg * 128],
                        oh_d[:, cb0 * 128 : (cb0 + cbg) * 128],
                    )
                    ohT = ohtpool.tile([128, CBG_MAX, 128], F8, tag="ohT")
                    nc.sync.dma_start(
                        ohT.rearrange("p a b -> p (a b)")[:, 0 : cbg * 128],
                        ohT_d[:, cb0 * 128 : (cb0 + cbg) * 128],
                    )

                    # per-edge a_dst via transposed one-hot x per-block vector
                    pa = ps_adst.tile([128, CBG_MAX], F32, space="PSUM")
                    for j in range(cbg):
                        nc.tensor.matmul(
                            pa[:, j : j + 1],
                            lhsT=ohT[:, j, :],
                            rhs=adst_sb[:, cb_block[cb0 + j] : cb_block[cb0 + j] + 1],
                            start=True,
                            stop=True,
                            skip_group_check=True,
                        )

                    # ee = exp(leakyrelu(|att_src|*t127 + a_dst')) and the
                    # scatter rhs [t*ee (128) | ee | pad], built per gather
                    # segment so DVE work overlaps the staggered DMA drains
                    # and gf releases incrementally
                    e1 = epool.tile([128, CBG_MAX], F32, tag="e1")
                    e3 = epool.tile([128, CBG_MAX], F32, tag="e3")
                    ee = epool.tile([128, CBG_MAX], F32, tag="ee")
                    gfe = gfepool.tile([128, CBG_MAX, 130], F16, tag="gfe")
                    for s0, s1, _hf in segs:
                        w = s1 - s0
                        nc.vector.scalar_tensor_tensor(
                            e1[:, s0:s1],
                            in0=gf[:, s0:s1, 127],
                            scalar=float(na1),
                            in1=pa[:, s0:s1],
                            op0=OP.mult,
                            op1=OP.add,
                        )
                        nc.vector.scalar_tensor_tensor(
                            e3[:, s0:s1],
                            in0=e1[:, s0:s1],
                            scalar=NEG_SLOPE,
                            in1=e1[:, s0:s1],
                            op0=OP.mult,
                            op1=OP.max,
                        )
                        nc.scalar.activation(ee[:, s0:s1], e3[:, s0:s1], AF.Exp)
                        nc.vector.tensor_tensor(
                            gfe[:, s0:s1, 0:128],
                            gf[:, s0:s1, :],
                            ee[:, s0:s1].to_broadcast([128, w, 128]),
                            OP.mult,
                        )
                        nc.vector.tensor_copy(gfe[:, s0:s1, 128], ee[:, s0:s1])

                    # scatter matmuls per block
                    lo_off = 0
                    hi_off = nlo
                    for b in blocks:
                        rows = 128 if b < NBLK - 1 else LAST_ROWS
                        cbs = list(range(lo_off, lo_off + tlo[b])) + list(
                            range(hi_off, hi_off + thi[b])
                        )
                        lo_off += tlo[b]
                        hi_off += thi[b]
                        ps = ps_sc.tile([128, 129], F32, space="PSUM")
                        for j, cb in enumerate(cbs):
                            nc.tensor.matmul(
                                ps[:, :],
                                lhsT=oh[:, cb, :],
                                rhs=gfe[:, cb, 0:129],
                                start=(j == 0),
                                stop=(j == len(cbs) - 1),
                            )
                        recip = blkpool.tile([128, 1], F32, tag="recip")
                        nc.vector.reciprocal(recip[:], ps[:, 128:129])
                        scaled = blkpool.tile([128, D], F32, tag="scaled")
                        nc.scalar.activation(
                            scaled[:], ps[:, 0:D], AF.Copy, scale=recip[:, 0:1]
                        )
                        ptp = ps_tp.tile([128, 128], F32, space="PSUM", tag="tp")
                        nc.tensor.transpose(ptp[:], scaled[:], ident_sb[:])
                        scaledT = blkpool.tile([128, D], F16, tag="scaledT")
                        nc.scalar.copy(scaledT[:], ptp[:])
                        po = ps_out.tile([128, 128], F32, space="PSUM", tag="tp")
                        nc.tensor.matmul(
                            po[:], lhsT=scaledT[:], rhs=q_sb[:], start=True, stop=False
                        )
                        nc.tensor.matmul(
                            po[:], lhsT=identh_sb[:], rhs=xh_tiles[b][:],
                            start=False, stop=True,
                        )
                        outt = blkpool.tile([128, D], F32, tag="outt")
                        nc.scalar.activation(outt[:], po[:], AF.Relu)
                        nc.sync.dma_start(
                            out_shard[b * 128 : b * 128 + rows, :], outt[:rows, :]
                        )

    nc.compile()
    return nc


def _wrap_idx(idx):
    """int16 index list -> dma_gather SBUF layout [128, len/16]:
    index i lives at partitions {16g + i%16: g in 0..7}, column i//16."""
    L = len(idx)
    assert L % 16 == 0
    w = idx.reshape(L // 16, 16).T.astype(np.int16)      # [16, L/16]
    return np.tile(w, (8, 1))                            # [128, L/16]


def _host_prep(x, edge_index, ln_gamma, ln_beta, W, att_src, att_dst, bias):
    """Fold parameters, build rotation Q, bucket edges. Numpy only."""
    Wt = W.T.astype(np.float64)
    G = ln_gamma.astype(np.float64)[:, None] * Wt          # [D, D]
    crow = ln_beta.astype(np.float64) @ Wt                 # [D]
    a1 = att_src.astype(np.float64)
    a2 = att_dst.astype(np.float64)
    na1 = float(np.linalg.norm(a1))
    v_dst = G @ a2
    c_dst = float(crow @ a2)
    kc = float(bias.astype(np.float64) @ a1)

    # orthonormal Q with row 127 = att_src direction (row 126: att_dst comp,
    # kept only so Q is deterministic/well-conditioned)
    q127 = a1 / na1
    u = a2 - (a2 @ q127) * q127
    nu = np.linalg.norm(u)
    if nu > 1e-12:
        q126 = u / nu
        P = np.eye(D) - np.outer(q127, q127) - np.outer(q126, q126)
        Uq, _, _ = np.linalg.svd(P)
        Q = np.vstack([Uq[:, :126].T, q126[None, :], q127[None, :]])
    else:
        P = np.eye(D) - np.outer(q127, q127)
        Uq, _, _ = np.linalg.svd(P)
        Q = np.vstack([Uq[:, :127].T, q127[None, :]])

    c2feat = crow + bias.astype(np.float64)
    wextq = np.zeros((D, 129), np.float16)
    wextq[:, 0:128] = (G @ Q.T).astype(np.float16)
    wextq[:, 128] = v_dst.astype(np.float16)
    c2 = np.zeros((129,), np.float32)
    c2[0:128] = (c2feat @ Q.T).astype(np.float32)
    c2[128] = c_dst - kc
    c2b = np.broadcast_to(c2, (128, 129)).copy()

    ident = np.eye(128, dtype=np.float32)
    qmat = Q.astype(np.float16)

    # edges + self loops, sorted by (core, block, src-half)
    src = np.concatenate([edge_index[0], np.arange(N, dtype=np.int64)]).astype(np.int64)
    dst = np.concatenate([edge_index[1], np.arange(N, dtype=np.int64)]).astype(np.int64)
    core = dst // SHARD
    local = dst - core * SHARD
    blk = local // 128
    half = (src >= HALF).astype(np.int64)
    key = ((core * NBLK + blk) * 2 + half)
    order = np.argsort(key, kind="stable")
    src, dst, key = src[order], dst[order], key[order]
    counts = np.bincount(key, minlength=NCORES * NBLK * 2).reshape(NCORES, NBLK, 2)
    tiles = -(-counts // 128)                              # ceil
    tlo = tuple(int(t) for t in tiles[:, :, 0].max(axis=0))
    thi = tuple(int(t) for t in tiles[:, :, 1].max(axis=0))
    CB = sum(tlo) + sum(thi)

    feat_idx = np.zeros((NCORES, CB * 128), np.int16)
    oh = np.zeros((NCORES, 128, CB, 128), np.uint8)
    ohT = np.zeros((NCORES, 128, CB, 128), np.uint8)

    starts = np.zeros(NCORES * NBLK * 2 + 1, np.int64)
    starts[1:] = np.cumsum(counts.reshape(-1))

    # cb offset of each (block, half) segment, same for every core
    seg_off = {}
    cb0 = 0
    for g0 in range(0, NBLK, GBLK):
        blocks = list(range(g0, min(NBLK, g0 + GBLK)))
        off = cb0
        for b in blocks:
            seg_off[(b, 0)] = off
            off += tlo[b]
        for b in blocks:
            seg_off[(b, 1)] = off
            off += thi[b]
        cb0 = off
    assert cb0 == CB

    for c in range(NCORES):
        for b in range(NBLK):
            for hf in range(2):
                gi = (c * NBLK + b) * 2 + hf
                s, e = starts[gi], starts[gi + 1]
                n = int(e - s)
                if n == 0:
                    continue
                off = seg_off[(b, hf)]
                k = np.arange(n) + off * 128
                fi = (src[s:e] - hf * HALF).astype(np.int16)
                feat_idx[c, k] = fi
                p = k % 128
                t = k // 128
                r = (dst[s:e] - (c * SHARD + b * 128)).astype(np.int64)
                oh[c, p, t, r] = 1
                ohT[c, r, t, p] = 1

    oh8 = oh.astype(ml_dtypes.float8_e4m3fn).reshape(NCORES, 128, CB * 128)
    ohT8 = ohT.astype(ml_dtypes.float8_e4m3fn).reshape(NCORES, 128, CB * 128)

    in_maps = []
    for c in range(NCORES):
        xs = np.zeros((PAD_SHARD, D), np.float32)
        xs[0:SHARD] = x[c * SHARD : (c + 1) * SHARD]
        in_maps.append(
            {
                "x_shard": xs,
                "wextq": wextq,
                "c2q": c2b,
                "ident": ident,
                "qmat": qmat,
                "feat_idx": _wrap_idx(feat_idx[c]),
                "oh_d": np.ascontiguousarray(oh8[c]),
                "ohT_d": np.ascontiguousarray(ohT8[c]),
            }
        )
    return tlo, thi, na1, in_maps


_PROGRAM_CACHE = {}


def kernel(x, edge_index, edge_attr, h, batch, ln_gamma, ln_beta, W, att_src,
           att_dst, bias):
    x = np.asarray(x, dtype=np.float32)
    edge_index = np.asarray(edge_index)
    h = np.asarray(h)
    ln_gamma = np.asarray(ln_gamma, dtype=np.float32)
    ln_beta = np.asarray(ln_beta, dtype=np.float32)
    W = np.asarray(W, dtype=np.float32)
    att_src = np.asarray(att_src, dtype=np.float32)
    att_dst = np.asarray(att_dst, dtype=np.float32)
    bias = np.asarray(bias, dtype=np.float32)

    tlo, thi, na1, in_maps = _host_prep(
        x, edge_index, ln_gamma, ln_beta, W, att_src, att_dst, bias
    )
    key = (tlo, thi, round(na1, 6))
    if key not in _PROGRAM_CACHE:
        _PROGRAM_CACHE[key] = _build_program(tlo, thi, na1)
    nc = _PROGRAM_CACHE[key]

    res = run_bass_kernel_spmd(nc, in_maps, core_ids=list(range(NCORES)))
    out = np.concatenate([res.results[c]["out_shard"] for c in range(NCORES)], axis=0)
    return out, h


# revision 27
# speedup vs baseline: 1.1209x; 1.0638x over previous
"""GAT layer (LayerNorm -> GATConv(heads=1) -> residual ReLU) on 8 trn2 NeuronCores.

Sharding: destination-node parallel. Each core owns N/8 contiguous nodes,
computes the node transform for its shard, AllGathers the transformed table,
then processes the edges whose destination lands in its shard.

Key design points (v2, rebuilt after profiling the 768B-row baseline):
- The node table is fp16, 256 B/row (the dma_gather minimum): a host-side
  orthonormal rotation Q puts att_src along coordinate 127, so the gathered
  row IS [rotated feats | a_src] with zero extra columns; the rotation is
  undone after the softmax-weighted scatter by one 128x128 matmul per dst
  block (Q is orthogonal, applied to the accumulated sums).
- No per-edge a_dst gather: a_dst per edge = ohT_cb^T @ adst_block via a
  1-column matmul per 128-edge column block, with one-hot tables streamed
  from the host as fp8 (exact 0/1).
- No DVE one-hot builds: the scatter matmul uses lhsT = host fp8 one-hot,
  rhs = gathered rows * ee (folded on ACT/DVE), with a constant ones column
  in the rhs producing the softmax denominator in the same matmul.
- Feature gathers round-robin over 4 SWDGE queues to overlap DMA drains.
"""

import numpy as np
import ml_dtypes

import concourse.bacc as bacc
import concourse.mybir as mybir
import concourse.tile as tile
from concourse.bass_utils import run_bass_kernel_spmd

F32 = mybir.dt.float32
F16 = mybir.dt.float16
F8 = mybir.dt.float8e4
I16 = mybir.dt.int16
AX = mybir.AxisListType
OP = mybir.AluOpType
AF = mybir.ActivationFunctionType

N = 50000
D = 128
E = 600000
NCORES = 8
SHARD = N // NCORES            # 6250
NBLK = (SHARD + 127) // 128    # 49 dst blocks per core
PAD_SHARD = NBLK * 128         # 6272
LAST_ROWS = SHARD - (NBLK - 1) * 128  # 106
HALF = 32768                   # int16 index split point for the global table
NEG_SLOPE = 0.2
LN_EPS = 1e-5
GBLK = 3                       # dst blocks per gather group
NQ = 4                         # SWDGE queues for gathers


def _build_program(tlo, thi, na1):
    """One SPMD program; per-core behaviour differs only through its inputs."""
    nc = bacc.Bacc("TRN2", num_devices=NCORES, debug=False, num_swdge_queues=NQ)

    CB = sum(tlo) + sum(thi)   # total column-blocks (tiles) per core

    x_shard = nc.dram_tensor("x_shard", [PAD_SHARD, D], F32, kind="ExternalInput")
    wextq = nc.dram_tensor("wextq", [D, 129], F16, kind="ExternalInput")
    c2q = nc.dram_tensor("c2q", [128, 129], F32, kind="ExternalInput")
    ident = nc.dram_tensor("ident", [128, 128], F32, kind="ExternalInput")
    qmat = nc.dram_tensor("qmat", [128, 128], F16, kind="ExternalInput")
    feat_idx = nc.dram_tensor("feat_idx", [128, CB * 8], I16, kind="ExternalInput")
    oh_d = nc.dram_tensor("oh_d", [128, CB * 128], F8, kind="ExternalInput")
    ohT_d = nc.dram_tensor("ohT_d", [128, CB * 128], F8, kind="ExternalInput")
    out_shard = nc.dram_tensor("out_shard", [SHARD, D], F32, kind="ExternalOutput")

    # group structure (static, identical on every core)
    groups = []
    cb0 = 0
    for g0 in range(0, NBLK, GBLK):
        blocks = list(range(g0, min(NBLK, g0 + GBLK)))
        nlo = sum(tlo[b] for b in blocks)
        nhi = sum(thi[b] for b in blocks)
        groups.append((blocks, cb0, nlo, nhi))
        cb0 += nlo + nhi
    assert cb0 == CB
    CBG_MAX = max(nlo + nhi for _, _, nlo, nhi in groups)

    # cb -> owning block (within its group), same order the host uses
    cb_block = [0] * CB
    for blocks, cb0g, nlo, nhi in groups:
        off = cb0g
        for b in blocks:
            for _ in range(tlo[b]):
                cb_block[off] = b
                off += 1
        for b in blocks:
            for _ in range(thi[b]):
                cb_block[off] = b
                off += 1

    with tile.TileContext(nc) as tc:
        with (
            tc.tile_pool(name="dram", bufs=1, space="DRAM") as dram,
            tc.tile_pool(name="consts", bufs=1) as cpool,
            tc.tile_pool(name="xres", bufs=1) as xpool,
            tc.tile_pool(name="xresh", bufs=1) as xhpool,
        ):
            xb_shard = dram.tile([SHARD, D], F16)
            xb_full = dram.tile([N, D], F16, addr_space="Shared")

            ident_sb = cpool.tile([128, 128], F32)
            nc.sync.dma_start(ident_sb[:], ident[:, :])
            q_sb = cpool.tile([128, 128], F16)
            nc.sync.dma_start(q_sb[:], qmat[:, :])
            wext_sb = cpool.tile([D, 129], F16)
            nc.sync.dma_start(wext_sb[:], wextq[:, :])
            identh_sb = cpool.tile([128, 128], F16)
            nc.scalar.copy(identh_sb[:], ident_sb[:])
            c2_sb = cpool.tile([128, 129], F32)
            nc.sync.dma_start(c2_sb[:], c2q[:, :])
            eps_sb = cpool.tile([128, 1], F32)
            nc.vector.memset(eps_sb[:], LN_EPS)
            fidx_sb = cpool.tile([128, CB * 8], I16)
            nc.sync.dma_start(fidx_sb[:], feat_idx[:, :])
            adst_sb = cpool.tile([128, NBLK], F16)

            x_tiles = []
            for i in range(NBLK):
                xt = xpool.tile([128, D], F32, tag=f"xres{i}")
                nc.sync.dma_start(xt[:], x_shard[i * 128 : (i + 1) * 128, :])
                x_tiles.append(xt)

            xh_tiles = []
            # ---------------- Phase A: node transform on own shard ---------
            with (
                tc.tile_pool(name="a_small", bufs=8) as spool,
                tc.tile_pool(name="a_xnp", bufs=3) as xnppool,
                tc.tile_pool(name="a_xnpT", bufs=3) as xnptpool,
                tc.tile_pool(name="a_xpe", bufs=3) as xpepool,
                tc.tile_pool(name="a_tb", bufs=3) as tbpool,
                tc.tile_pool(name="a_ps_t", bufs=2, space="PSUM") as psa,
                tc.tile_pool(name="a_ps_m", bufs=2, space="PSUM") as psb,
            ):
                for i in range(NBLK):
                    xt = x_tiles[i]
                    rows = 128 if i < NBLK - 1 else LAST_ROWS
                    stats = spool.tile([128, 6], F32, tag="stats")
                    nc.vector.bn_stats(stats[:], xt[:])
                    mv = spool.tile([128, 2], F32, tag="mv")
                    nc.vector.bn_aggr(mv[:], stats[:])
                    std = spool.tile([128, 1], F32, tag="std")
                    nc.scalar.activation(
                        std[:], mv[:, 1:2], AF.Sqrt, bias=eps_sb[:, 0:1]
                    )
                    rstd = spool.tile([128, 1], F32, tag="rstd")
                    nc.vector.reciprocal(rstd[:], std[:])
                    xnp = xnppool.tile([128, D], F16)
                    nc.vector.tensor_scalar(
                        xnp[:], xt[:], mv[:, 0:1], rstd[:, 0:1], OP.subtract, OP.mult
                    )
                    xh = xhpool.tile([128, D], F16, tag=f"xh{i}")
                    nc.scalar.copy(xh[:], xt[:])
                    xh_tiles.append(xh)
                    pt = psa.tile([128, 128], F16, space="PSUM")
                    nc.tensor.transpose(pt[:], xnp[:], identh_sb[:])
                    xnpT = xnptpool.tile([128, 128], F16)
                    nc.scalar.copy(xnpT[:], pt[:])
                    pm = psb.tile([128, 129], F32, space="PSUM")
                    nc.tensor.matmul(
                        pm[:], lhsT=xnpT[:], rhs=wext_sb[:], start=True, stop=True
                    )
                    xpe = xpepool.tile([128, 129], F32)
                    nc.vector.tensor_tensor(xpe[:], pm[:], c2_sb[:], OP.add)
                    tb = tbpool.tile([128, D], F16, tag="tb")
                    nc.scalar.copy(tb[:], xpe[:, 0:128])
                    nc.sync.dma_start(
                        xb_shard[i * 128 : i * 128 + rows, :], tb[:rows, :]
                    )
                    nc.scalar.copy(adst_sb[:, i : i + 1], xpe[:, 128:129])

            nc.gpsimd.collective_compute(
                "AllGather",
                OP.bypass,
                replica_groups=[list(range(NCORES))],
                ins=[xb_shard[:, :]],
                outs=[xb_full[:, :]],
            )

            # ---------------- Phase B: edge aggregation --------------------
            with (
                tc.tile_pool(name="b_g", bufs=5) as gpool,
                tc.tile_pool(name="b_oh", bufs=3) as ohpool,
                tc.tile_pool(name="b_ohT", bufs=3) as ohtpool,
                tc.tile_pool(name="b_e", bufs=3) as epool,
                tc.tile_pool(name="b_gfe", bufs=3) as gfepool,
                tc.tile_pool(name="b_blk", bufs=4) as blkpool,
                tc.tile_pool(name="b_ps_a", bufs=2, space="PSUM") as ps_adst,
                tc.tile_pool(name="b_ps_s", bufs=3, space="PSUM") as ps_sc,
                tc.tile_pool(name="b_ps_t", bufs=2, space="PSUM") as ps_tp,
            ):
                ps_out = ps_tp
                qctr = 0
                for gi, (blocks, cb0, nlo, nhi) in enumerate(groups):
                    cbg = nlo + nhi
                    gf = gpool.tile([128, CBG_MAX, 128], F16, tag="gf")
                    # split each half-table gather in two on separate SWDGE
                    # queues so their DMA drains overlap
                    segs = []
                    if nlo:
                        h1 = (nlo + 1) // 2
                        segs += [(0, h1, 0), (h1, nlo, 0)] if nlo > 1 else [(0, nlo, 0)]
                    if nhi:
                        h2 = (nhi + 1) // 2
                        segs += (
                            [(nlo, nlo + h2, 1), (nlo + h2, cbg, 1)]
                            if nhi > 1
                            else [(nlo, cbg, 1)]
                        )
                    for s0, s1, hf in segs:
                        nc.gpsimd.dma_gather(
                            out_ap=gf[:, s0:s1, :],
                            in_ap=xb_full[0:HALF, :] if hf == 0 else xb_full[HALF:N, :],
                            idxs_ap=fidx_sb[:, (cb0 + s0) * 8 : (cb0 + s1) * 8],
                            num_idxs=(s1 - s0) * 128,
                            num_idxs_reg=(s1 - s0) * 128,
                            elem_size=128,
                            single_packet=False,
                            queue_num=qctr % NQ,
                        )
                        qctr += 1
                    oh = ohpool.tile([128, CBG_MAX, 128], F8, tag="oh")
                    nc.sync.dma_start(
                        oh.rearrange("p a b -> p (a b)")[:, 0 : cbg * 128],
                        oh_d[:, cb0 * 128 : (cb0 + cbg) * 128],
                    )
                    ohT = ohtpool.tile([128, CBG_MAX, 128], F8, tag="ohT")
                    nc.sync.dma_start(
                        ohT.rearrange("p a b -> p (a b)")[:, 0 : cbg * 128],
                        ohT_d[:, cb0 * 128 : (cb0 + cbg) * 128],
                    )

                    # per-edge a_dst via transposed one-hot x per-block vector
                    pa = ps_adst.tile([128, CBG_MAX], F32, space="PSUM")
                    for j in range(cbg):
                        nc.tensor.matmul(
                            pa[:, j : j + 1],
                            lhsT=ohT[:, j, :],
                            rhs=adst_sb[:, cb_block[cb0 + j] : cb_block[cb0 + j] + 1],
                            start=True,
                            stop=True,
                            skip_group_check=True,
                        )

                    # ee = exp(leakyrelu(|att_src|*t127 + a_dst')) and the
                    # scatter rhs [t*ee (128) | ee | pad], built per gather
                    # segment so DVE work overlaps the staggered DMA drains
                    # and gf releases incrementally
                    e1 = epool.tile([128, CBG_MAX], F32, tag="e1")
                    e3 = epool.tile([128, CBG_MAX], F32, tag="e3")
                    ee = epool.tile([128, CBG_MAX], F32, tag="ee")
                    gfe = gfepool.tile([128, CBG_MAX, 130], F16, tag="gfe")
                    for s0, s1, _hf in segs:
                        w = s1 - s0
                        nc.vector.scalar_tensor_tensor(
                            e1[:, s0:s1],
                            in0=gf[:, s0:s1, 127],
                            scalar=float(na1),
                            in1=pa[:, s0:s1],
                            op0=OP.mult,
                            op1=OP.add,
                        )
                        nc.vector.scalar_tensor_tensor(
                            e3[:, s0:s1],
                            in0=e1[:, s0:s1],
                            scalar=NEG_SLOPE,
                            in1=e1[:, s0:s1],
                            op0=OP.mult,
                            op1=OP.max,
                        )
                        nc.scalar.activation(ee[:, s0:s1], e3[:, s0:s1], AF.Exp)
                        nc.vector.tensor_tensor(
                            gfe[:, s0:s1, 0:128],
                            gf[:, s0:s1, :],
                            ee[:, s0:s1].to_broadcast([128, w, 128]),
                            OP.mult,
                        )
                        nc.vector.tensor_copy(gfe[:, s0:s1, 128], ee[:, s0:s1])

                    # scatter matmuls per block
                    lo_off = 0
                    hi_off = nlo
                    for b in blocks:
                        rows = 128 if b < NBLK - 1 else LAST_ROWS
                        cbs = list(range(lo_off, lo_off + tlo[b])) + list(
                            range(hi_off, hi_off + thi[b])
                        )
                        lo_off += tlo[b]
                        hi_off += thi[b]
                        ps = ps_sc.tile([128, 129], F32, space="PSUM")
                        for j, cb in enumerate(cbs):
                            nc.tensor.matmul(
                                ps[:, :],
                                lhsT=oh[:, cb, :],
                                rhs=gfe[:, cb, 0:129],
                                start=(j == 0),
                                stop=(j == len(cbs) - 1),
                            )
                        recip = blkpool.tile([128, 1], F32, tag="recip")
                        nc.vector.reciprocal(recip[:], ps[:, 128:129])
                        scaled = blkpool.tile([128, D], F32, tag="scaled")
                        nc.scalar.activation(
                            scaled[:], ps[:, 0:D], AF.Copy, scale=recip[:, 0:1]
                        )
                        ptp = ps_tp.tile([128, 128], F32, space="PSUM", tag="tp")
                        nc.tensor.transpose(ptp[:], scaled[:], ident_sb[:])
                        scaledT = blkpool.tile([128, D], F16, tag="scaledT")
                        nc.scalar.copy(scaledT[:], ptp[:])
                        po = ps_out.tile([128, 128], F32, space="PSUM", tag="tp")
                        nc.tensor.matmul(
                            po[:], lhsT=scaledT[:], rhs=q_sb[:], start=True, stop=False
                        )
                        nc.tensor.matmul(
                            po[:], lhsT=identh_sb[:], rhs=xh_tiles[b][:],
                            start=False, stop=True,
                        )
                        outt = blkpool.tile([128, D], F32, tag="outt")
                        nc.scalar.activation(outt[:], po[:], AF.Relu)
                        nc.sync.dma_start(
                            out_shard[b * 128 : b * 128 + rows, :], outt[:rows, :]
                        )

    nc.compile()
    return nc


def _wrap_idx(idx):
    """int16 index list -> dma_gather SBUF layout [128, len/16]:
    index i lives at partitions {16g + i%16: g in 0..7}, column i//16."""
    L = len(idx)
    assert L % 16 == 0
    w = idx.reshape(L // 16, 16).T.astype(np.int16)      # [16, L/16]
    return np.tile(w, (8, 1))                            # [128, L/16]


def _host_prep(x, edge_index, ln_gamma, ln_beta, W, att_src, att_dst, bias):
    """Fold parameters, build rotation Q, bucket edges. Numpy only."""
    Wt = W.T.astype(np.float64)
    G = ln_gamma.astype(np.float64)[:, None] * Wt          # [D, D]
    crow = ln_beta.astype(np.float64) @ Wt                 # [D]
    a1 = att_src.astype(np.float64)
    a2 = att_dst.astype(np.float64)
    na1 = float(np.linalg.norm(a1))
    v_dst = G @ a2
    c_dst = float(crow @ a2)
    kc = float(bias.astype(np.float64) @ a1)

    # orthonormal Q with row 127 = att_src direction (row 126: att_dst comp,
    # kept only so Q is deterministic/well-conditioned)
    q127 = a1 / na1
    u = a2 - (a2 @ q127) * q127
    nu = np.linalg.norm(u)
    if nu > 1e-12:
        q126 = u / nu
        P = np.eye(D) - np.outer(q127, q127) - np.outer(q126, q126)
        Uq, _, _ = np.linalg.svd(P)
        Q = np.vstack([Uq[:, :126].T, q126[None, :], q127[None, :]])
    else:
        P = np.eye(D) - np.outer(q127, q127)
        Uq, _, _ = np.linalg.svd(P)
        Q = np.vstack([Uq[:, :127].T, q127[None, :]])

    c2feat = crow + bias.astype(np.float64)
    wextq = np.zeros((D, 129), np.float16)
    wextq[:, 0:128] = (G @ Q.T).astype(np.float16)
    wextq[:, 128] = v_dst.astype(np.float16)
    c2 = np.zeros((129,), np.float32)
    c2[0:128] = (c2feat @ Q.T).astype(np.float32)
    c2[128] = c_dst - kc
    c2b = np.broadcast_to(c2, (128, 129)).copy()

    ident = np.eye(128, dtype=np.float32)
    qmat = Q.astype(np.float16)

    # edges + self loops, sorted by (core, block, src-half)
    src = np.concatenate([edge_index[0], np.arange(N, dtype=np.int64)]).astype(np.int64)
    dst = np.concatenate([edge_index[1], np.arange(N, dtype=np.int64)]).astype(np.int64)
    core = dst // SHARD
    local = dst - core * SHARD
    blk = local // 128
    half = (src >= HALF).astype(np.int64)
    key = ((core * NBLK + blk) * 2 + half)
    order = np.argsort(key, kind="stable")
    src, dst, key = src[order], dst[order], key[order]
    counts = np.bincount(key, minlength=NCORES * NBLK * 2).reshape(NCORES, NBLK, 2)
    tiles = -(-counts // 128)                              # ceil
    tlo = tuple(int(t) for t in tiles[:, :, 0].max(axis=0))
    thi = tuple(int(t) for t in tiles[:, :, 1].max(axis=0))
    CB = sum(tlo) + sum(thi)

    feat_idx = np.zeros((NCORES, CB * 128), np.int16)
    oh = np.zeros((NCORES, 128, CB, 128), np.uint8)
    ohT = np.zeros((NCORES, 128, CB, 128), np.uint8)

    starts = np.zeros(NCORES * NBLK * 2 + 1, np.int64)
    starts[1:] = np.cumsum(counts.reshape(-1))

    # cb offset of each (block, half) segment, same for every core
    seg_off = {}
    cb0 = 0
    for g0 in range(0, NBLK, GBLK):
        blocks = list(range(g0, min(NBLK, g0 + GBLK)))
        off = cb0
        for b in blocks:
            seg_off[(b, 0)] = off
            off += tlo[b]
        for b in blocks:
            seg_off[(b, 1)] = off
            off += thi[b]
        cb0 = off
    assert cb0 == CB

    for c in range(NCORES):
        for b in range(NBLK):
            for hf in range(2):
                gi = (c * NBLK + b) * 2 + hf
                s, e = starts[gi], starts[gi + 1]
                n = int(e - s)
                if n == 0:
                    continue
                off = seg_off[(b, hf)]
                k = np.arange(n) + off * 128
                fi = (src[s:e] - hf * HALF).astype(np.int16)
                feat_idx[c, k] = fi
                p = k % 128
                t = k // 128
                r = (dst[s:e] - (c * SHARD + b * 128)).astype(np.int64)
                oh[c, p, t, r] = 1
                ohT[c, r, t, p] = 1

    oh8 = oh.astype(ml_dtypes.float8_e4m3fn).reshape(NCORES, 128, CB * 128)
    ohT8 = ohT.astype(ml_dtypes.float8_e4m3fn).reshape(NCORES, 128, CB * 128)

    in_maps = []
    for c in range(NCORES):
        xs = np.zeros((PAD_SHARD, D), np.float32)
        xs[0:SHARD] = x[c * SHARD : (c + 1) * SHARD]
        in_maps.append(
            {
                "x_shard": xs,
                "wextq": wextq,
                "c2q": c2b,
                "ident": ident,
                "qmat": qmat,
                "feat_idx": _wrap_idx(feat_idx[c]),
                "oh_d": np.ascontiguousarray(oh8[c]),
                "ohT_d": np.ascontiguousarray(ohT8[c]),
            }
        )
    return tlo, thi, na1, in_maps


_PROGRAM_CACHE = {}


def kernel(x, edge_index, edge_attr, h, batch, ln_gamma, ln_beta, W, att_src,
           att_dst, bias):
    x = np.asarray(x, dtype=np.float32)
    edge_index = np.asarray(edge_index)
    h = np.asarray(h)
    ln_gamma = np.asarray(ln_gamma, dtype=np.float32)
    ln_beta = np.asarray(ln_beta, dtype=np.float32)
    W = np.asarray(W, dtype=np.float32)
    att_src = np.asarray(att_src, dtype=np.float32)
    att_dst = np.asarray(att_dst, dtype=np.float32)
    bias = np.asarray(bias, dtype=np.float32)

    tlo, thi, na1, in_maps = _host_prep(
        x, edge_index, ln_gamma, ln_beta, W, att_src, att_dst, bias
    )
    key = (tlo, thi, round(na1, 6))
    if key not in _PROGRAM_CACHE:
        _PROGRAM_CACHE[key] = _build_program(tlo, thi, na1)
    nc = _PROGRAM_CACHE[key]

    res = run_bass_kernel_spmd(nc, in_maps, core_ids=list(range(NCORES)))
    out = np.concatenate([res.results[c]["out_shard"] for c in range(NCORES)], axis=0)
    return out, h


# revision 28
# speedup vs baseline: 1.1388x; 1.0160x over previous
"""GAT layer (LayerNorm -> GATConv(heads=1) -> residual ReLU) on 8 trn2 NeuronCores.

Sharding: destination-node parallel. Each core owns N/8 contiguous nodes,
computes the node transform for its shard, AllGathers the transformed table,
then processes the edges whose destination lands in its shard.

Key design points (v2, rebuilt after profiling the 768B-row baseline):
- The node table is fp16, 256 B/row (the dma_gather minimum): a host-side
  orthonormal rotation Q puts att_src along coordinate 127, so the gathered
  row IS [rotated feats | a_src] with zero extra columns; the rotation is
  undone after the softmax-weighted scatter by one 128x128 matmul per dst
  block (Q is orthogonal, applied to the accumulated sums).
- No per-edge a_dst gather: a_dst per edge = ohT_cb^T @ adst_block via a
  1-column matmul per 128-edge column block, with one-hot tables streamed
  from the host as fp8 (exact 0/1).
- No DVE one-hot builds: the scatter matmul uses lhsT = host fp8 one-hot,
  rhs = gathered rows * ee (folded on ACT/DVE), with a constant ones column
  in the rhs producing the softmax denominator in the same matmul.
- Feature gathers round-robin over 4 SWDGE queues to overlap DMA drains.
"""

import numpy as np
import ml_dtypes

import concourse.bacc as bacc
import concourse.mybir as mybir
import concourse.tile as tile
from concourse.bass_utils import run_bass_kernel_spmd

F32 = mybir.dt.float32
F16 = mybir.dt.float16
F8 = mybir.dt.float8e4
I16 = mybir.dt.int16
AX = mybir.AxisListType
OP = mybir.AluOpType
AF = mybir.ActivationFunctionType

N = 50000
D = 128
E = 600000
NCORES = 8
SHARD = N // NCORES            # 6250
NBLK = (SHARD + 127) // 128    # 49 dst blocks per core
PAD_SHARD = NBLK * 128         # 6272
LAST_ROWS = SHARD - (NBLK - 1) * 128  # 106
HALF = 32768                   # int16 index split point for the global table
NEG_SLOPE = 0.2
LN_EPS = 1e-5
GBLK = 3                       # dst blocks per gather group
NQ = 4                         # SWDGE queues for gathers


def _build_program(tlo, thi, na1):
    """One SPMD program; per-core behaviour differs only through its inputs."""
    nc = bacc.Bacc("TRN2", num_devices=NCORES, debug=False, num_swdge_queues=NQ)

    CB = sum(tlo) + sum(thi)   # total column-blocks (tiles) per core

    x_shard = nc.dram_tensor("x_shard", [PAD_SHARD, D], F32, kind="ExternalInput")
    wextq = nc.dram_tensor("wextq", [D, 129], F16, kind="ExternalInput")
    c2q = nc.dram_tensor("c2q", [128, 129], F32, kind="ExternalInput")
    ident = nc.dram_tensor("ident", [128, 128], F32, kind="ExternalInput")
    qmat = nc.dram_tensor("qmat", [128, 128], F16, kind="ExternalInput")
    feat_idx = nc.dram_tensor("feat_idx", [128, CB * 8], I16, kind="ExternalInput")
    oh_d = nc.dram_tensor("oh_d", [128, CB * 128], F8, kind="ExternalInput")
    ohT_d = nc.dram_tensor("ohT_d", [128, CB * 128], F8, kind="ExternalInput")
    out_shard = nc.dram_tensor("out_shard", [SHARD, D], F32, kind="ExternalOutput")

    # group structure (static, identical on every core)
    groups = []
    cb0 = 0
    for g0 in range(0, NBLK, GBLK):
        blocks = list(range(g0, min(NBLK, g0 + GBLK)))
        nlo = sum(tlo[b] for b in blocks)
        nhi = sum(thi[b] for b in blocks)
        groups.append((blocks, cb0, nlo, nhi))
        cb0 += nlo + nhi
    assert cb0 == CB
    CBG_MAX = max(nlo + nhi for _, _, nlo, nhi in groups)

    # cb -> owning block (within its group), same order the host uses
    cb_block = [0] * CB
    for blocks, cb0g, nlo, nhi in groups:
        off = cb0g
        for b in blocks:
            for _ in range(tlo[b]):
                cb_block[off] = b
                off += 1
        for b in blocks:
            for _ in range(thi[b]):
                cb_block[off] = b
                off += 1

    with tile.TileContext(nc) as tc:
        with (
            tc.tile_pool(name="dram", bufs=1, space="DRAM") as dram,
            tc.tile_pool(name="consts", bufs=1) as cpool,
            tc.tile_pool(name="xres", bufs=1) as xpool,
            tc.tile_pool(name="xresh", bufs=1) as xhpool,
        ):
            xb_shard = dram.tile([SHARD, D], F16)
            xb_full = dram.tile([N, D], F16, addr_space="Shared")

            ident_sb = cpool.tile([128, 128], F32)
            nc.sync.dma_start(ident_sb[:], ident[:, :])
            q_sb = cpool.tile([128, 128], F16)
            nc.sync.dma_start(q_sb[:], qmat[:, :])
            wext_sb = cpool.tile([D, 129], F16)
            nc.sync.dma_start(wext_sb[:], wextq[:, :])
            identh_sb = cpool.tile([128, 128], F16)
            nc.scalar.copy(identh_sb[:], ident_sb[:])
            c2_sb = cpool.tile([128, 129], F32)
            nc.sync.dma_start(c2_sb[:], c2q[:, :])
            eps_sb = cpool.tile([128, 1], F32)
            nc.vector.memset(eps_sb[:], LN_EPS)
            fidx_sb = cpool.tile([128, CB * 8], I16)
            nc.sync.dma_start(fidx_sb[:], feat_idx[:, :])
            adst_sb = cpool.tile([128, NBLK], F16)

            x_tiles = []
            for i in range(NBLK):
                xt = xpool.tile([128, D], F32, tag=f"xres{i}")
                nc.sync.dma_start(xt[:], x_shard[i * 128 : (i + 1) * 128, :])
                x_tiles.append(xt)

            xh_tiles = []
            # ---------------- Phase A: node transform on own shard ---------
            with (
                tc.tile_pool(name="a_small", bufs=8) as spool,
                tc.tile_pool(name="a_xnp", bufs=3) as xnppool,
                tc.tile_pool(name="a_xnpT", bufs=3) as xnptpool,
                tc.tile_pool(name="a_xpe", bufs=3) as xpepool,
                tc.tile_pool(name="a_tb", bufs=3) as tbpool,
                tc.tile_pool(name="a_ps_t", bufs=2, space="PSUM") as psa,
                tc.tile_pool(name="a_ps_m", bufs=2, space="PSUM") as psb,
            ):
                for i in range(NBLK):
                    xt = x_tiles[i]
                    rows = 128 if i < NBLK - 1 else LAST_ROWS
                    stats = spool.tile([128, 6], F32, tag="stats")
                    nc.vector.bn_stats(stats[:], xt[:])
                    mv = spool.tile([128, 2], F32, tag="mv")
                    nc.vector.bn_aggr(mv[:], stats[:])
                    std = spool.tile([128, 1], F32, tag="std")
                    nc.scalar.activation(
                        std[:], mv[:, 1:2], AF.Sqrt, bias=eps_sb[:, 0:1]
                    )
                    rstd = spool.tile([128, 1], F32, tag="rstd")
                    nc.vector.reciprocal(rstd[:], std[:])
                    xnp = xnppool.tile([128, D], F16)
                    nc.vector.tensor_scalar(
                        xnp[:], xt[:], mv[:, 0:1], rstd[:, 0:1], OP.subtract, OP.mult
                    )
                    xh = xhpool.tile([128, D], F16, tag=f"xh{i}")
                    nc.scalar.copy(xh[:], xt[:])
                    xh_tiles.append(xh)
                    pt = psa.tile([128, 128], F16, space="PSUM")
                    nc.tensor.transpose(pt[:], xnp[:], identh_sb[:])
                    xnpT = xnptpool.tile([128, 128], F16)
                    nc.scalar.copy(xnpT[:], pt[:])
                    pm = psb.tile([128, 129], F32, space="PSUM")
                    nc.tensor.matmul(
                        pm[:], lhsT=xnpT[:], rhs=wext_sb[:], start=True, stop=True
                    )
                    xpe = xpepool.tile([128, 129], F32)
                    nc.vector.tensor_tensor(xpe[:], pm[:], c2_sb[:], OP.add)
                    tb = tbpool.tile([128, D], F16, tag="tb")
                    nc.scalar.copy(tb[:], xpe[:, 0:128])
                    nc.sync.dma_start(
                        xb_shard[i * 128 : i * 128 + rows, :], tb[:rows, :]
                    )
                    nc.scalar.copy(adst_sb[:, i : i + 1], xpe[:, 128:129])

            nc.gpsimd.collective_compute(
                "AllGather",
                OP.bypass,
                replica_groups=[list(range(NCORES))],
                ins=[xb_shard[:, :]],
                outs=[xb_full[:, :]],
            )

            # ---------------- Phase B: edge aggregation --------------------
            with (
                tc.tile_pool(name="b_g", bufs=5) as gpool,
                tc.tile_pool(name="b_oh", bufs=3) as ohpool,
                tc.tile_pool(name="b_ohT", bufs=3) as ohtpool,
                tc.tile_pool(name="b_e", bufs=3) as epool,
                tc.tile_pool(name="b_gfe", bufs=3) as gfepool,
                tc.tile_pool(name="b_blk", bufs=4) as blkpool,
                tc.tile_pool(name="b_ps_a", bufs=2, space="PSUM") as ps_adst,
                tc.tile_pool(name="b_ps_s", bufs=3, space="PSUM") as ps_sc,
                tc.tile_pool(name="b_ps_t", bufs=2, space="PSUM") as ps_tp,
            ):
                ps_out = ps_tp
                qctr = 0
                for gi, (blocks, cb0, nlo, nhi) in enumerate(groups):
                    cbg = nlo + nhi
                    gf = gpool.tile([128, CBG_MAX, 128], F16, tag="gf")
                    # split each half-table gather in two on separate SWDGE
                    # queues so their DMA drains overlap
                    segs = []
                    if nlo:
                        h1 = (nlo + 1) // 2
                        segs += [(0, h1, 0), (h1, nlo, 0)] if nlo > 1 else [(0, nlo, 0)]
                    if nhi:
                        h2 = (nhi + 1) // 2
                        segs += (
                            [(nlo, nlo + h2, 1), (nlo + h2, cbg, 1)]
                            if nhi > 1
                            else [(nlo, cbg, 1)]
                        )
                    for s0, s1, hf in segs:
                        nc.gpsimd.dma_gather(
                            out_ap=gf[:, s0:s1, :],
                            in_ap=xb_full[0:HALF, :] if hf == 0 else xb_full[HALF:N, :],
                            idxs_ap=fidx_sb[:, (cb0 + s0) * 8 : (cb0 + s1) * 8],
                            num_idxs=(s1 - s0) * 128,
                            num_idxs_reg=(s1 - s0) * 128,
                            elem_size=128,
                            single_packet=False,
                            queue_num=qctr % NQ,
                        )
                        qctr += 1
                    oh = ohpool.tile([128, CBG_MAX, 128], F8, tag="oh")
                    nc.sync.dma_start(
                        oh.rearrange("p a b -> p (a b)")[:, 0 : cbg * 128],
                        oh_d[:, cb0 * 128 : (cb0 + cbg) * 128],
                    )
                    ohT = ohtpool.tile([128, CBG_MAX, 128], F8, tag="ohT")
                    nc.sync.dma_start(
                        ohT.rearrange("p a b -> p (a b)")[:, 0 : cbg * 128],
                        ohT_d[:, cb0 * 128 : (cb0 + cbg) * 128],
                    )

                    # per-edge a_dst via transposed one-hot x per-block vector
                    pa = ps_adst.tile([128, CBG_MAX], F32, space="PSUM")
                    for j in range(cbg):
                        nc.tensor.matmul(
                            pa[:, j : j + 1],
                            lhsT=ohT[:, j, :],
                            rhs=adst_sb[:, cb_block[cb0 + j] : cb_block[cb0 + j] + 1],
                            start=True,
                            stop=True,
                            skip_group_check=True,
                        )

                    # ee = exp(leakyrelu(|att_src|*t127 + a_dst')), then the
                    # scatter rhs [t*ee (128) | ee | pad] in one batched
                    # multiply + one strided column copy
                    e1 = epool.tile([128, CBG_MAX], F32, tag="e1")
                    nc.vector.scalar_tensor_tensor(
                        e1[:, 0:cbg],
                        in0=gf[:, 0:cbg, 127],
                        scalar=float(na1),
                        in1=pa[:, 0:cbg],
                        op0=OP.mult,
                        op1=OP.add,
                    )
                    e3 = epool.tile([128, CBG_MAX], F32, tag="e3")
                    nc.vector.scalar_tensor_tensor(
                        e3[:, 0:cbg],
                        in0=e1[:, 0:cbg],
                        scalar=NEG_SLOPE,
                        in1=e1[:, 0:cbg],
                        op0=OP.mult,
                        op1=OP.max,
                    )
                    ee = epool.tile([128, CBG_MAX], F32, tag="ee")
                    nc.scalar.activation(ee[:, 0:cbg], e3[:, 0:cbg], AF.Exp)
                    gfe = gfepool.tile([128, CBG_MAX, 130], F16, tag="gfe")
                    nc.vector.tensor_tensor(
                        gfe[:, 0:cbg, 0:128],
                        gf[:, 0:cbg, :],
                        ee[:, 0:cbg].to_broadcast([128, cbg, 128]),
                        OP.mult,
                    )
                    nc.vector.tensor_copy(gfe[:, 0:cbg, 128], ee[:, 0:cbg])

                    # scatter matmuls per block
                    lo_off = 0
                    hi_off = nlo
                    for b in blocks:
                        rows = 128 if b < NBLK - 1 else LAST_ROWS
                        cbs = list(range(lo_off, lo_off + tlo[b])) + list(
                            range(hi_off, hi_off + thi[b])
                        )
                        lo_off += tlo[b]
                        hi_off += thi[b]
                        ps = ps_sc.tile([128, 129], F32, space="PSUM")
                        for j, cb in enumerate(cbs):
                            nc.tensor.matmul(
                                ps[:, :],
                                lhsT=oh[:, cb, :],
                                rhs=gfe[:, cb, 0:129],
                                start=(j == 0),
                                stop=(j == len(cbs) - 1),
                            )
                        recip = blkpool.tile([128, 1], F32, tag="recip")
                        nc.vector.reciprocal(recip[:], ps[:, 128:129])
                        scaled = blkpool.tile([128, D], F32, tag="scaled")
                        nc.scalar.activation(
                            scaled[:], ps[:, 0:D], AF.Copy, scale=recip[:, 0:1]
                        )
                        ptp = ps_tp.tile([128, 128], F32, space="PSUM", tag="tp")
                        nc.tensor.transpose(ptp[:], scaled[:], ident_sb[:])
                        scaledT = blkpool.tile([128, D], F16, tag="scaledT")
                        nc.scalar.copy(scaledT[:], ptp[:])
                        po = ps_out.tile([128, 128], F32, space="PSUM", tag="tp")
                        nc.tensor.matmul(
                            po[:], lhsT=scaledT[:], rhs=q_sb[:], start=True, stop=False
                        )
                        nc.tensor.matmul(
                            po[:], lhsT=identh_sb[:], rhs=xh_tiles[b][:],
                            start=False, stop=True,
                        )
                        outt = blkpool.tile([128, D], F32, tag="outt")
                        nc.scalar.activation(outt[:], po[:], AF.Relu)
                        nc.sync.dma_start(
                            out_shard[b * 128 : b * 128 + rows, :], outt[:rows, :]
                        )

    nc.compile()
    return nc


def _wrap_idx(idx):
    """int16 index list -> dma_gather SBUF layout [128, len/16]:
    index i lives at partitions {16g + i%16: g in 0..7}, column i//16."""
    L = len(idx)
    assert L % 16 == 0
    w = idx.reshape(L // 16, 16).T.astype(np.int16)      # [16, L/16]
    return np.tile(w, (8, 1))                            # [128, L/16]


def _host_prep(x, edge_index, ln_gamma, ln_beta, W, att_src, att_dst, bias):
    """Fold parameters, build rotation Q, bucket edges. Numpy only."""
    Wt = W.T.astype(np.float64)
    G = ln_gamma.astype(np.float64)[:, None] * Wt          # [D, D]
    crow = ln_beta.astype(np.float64) @ Wt                 # [D]
    a1 = att_src.astype(np.float64)
    a2 = att_dst.astype(np.float64)
    na1 = float(np.linalg.norm(a1))
    v_dst = G @ a2
    c_dst = float(crow @ a2)
    kc = float(bias.astype(np.float64) @ a1)

    # orthonormal Q with row 127 = att_src direction (row 126: att_dst comp,
    # kept only so Q is deterministic/well-conditioned)
    q127 = a1 / na1
    u = a2 - (a2 @ q127) * q127
    nu = np.linalg.norm(u)
    if nu > 1e-12:
        q126 = u / nu
        P = np.eye(D) - np.outer(q127, q127) - np.outer(q126, q126)
        Uq, _, _ = np.linalg.svd(P)
        Q = np.vstack([Uq[:, :126].T, q126[None, :], q127[None, :]])
    else:
        P = np.eye(D) - np.outer(q127, q127)
        Uq, _, _ = np.linalg.svd(P)
        Q = np.vstack([Uq[:, :127].T, q127[None, :]])

    c2feat = crow + bias.astype(np.float64)
    wextq = np.zeros((D, 129), np.float16)
    wextq[:, 0:128] = (G @ Q.T).astype(np.float16)
    wextq[:, 128] = v_dst.astype(np.float16)
    c2 = np.zeros((129,), np.float32)
    c2[0:128] = (c2feat @ Q.T).astype(np.float32)
    c2[128] = c_dst - kc
    c2b = np.broadcast_to(c2, (128, 129)).copy()

    ident = np.eye(128, dtype=np.float32)
    qmat = Q.astype(np.float16)

    # edges + self loops, sorted by (core, block, src-half)
    src = np.concatenate([edge_index[0], np.arange(N, dtype=np.int64)]).astype(np.int64)
    dst = np.concatenate([edge_index[1], np.arange(N, dtype=np.int64)]).astype(np.int64)
    core = dst // SHARD
    local = dst - core * SHARD
    blk = local // 128
    half = (src >= HALF).astype(np.int64)
    key = ((core * NBLK + blk) * 2 + half)
    order = np.argsort(key, kind="stable")
    src, dst, key = src[order], dst[order], key[order]
    counts = np.bincount(key, minlength=NCORES * NBLK * 2).reshape(NCORES, NBLK, 2)
    tiles = -(-counts // 128)                              # ceil
    tlo = tuple(int(t) for t in tiles[:, :, 0].max(axis=0))
    thi = tuple(int(t) for t in tiles[:, :, 1].max(axis=0))
    CB = sum(tlo) + sum(thi)

    feat_idx = np.zeros((NCORES, CB * 128), np.int16)
    oh = np.zeros((NCORES, 128, CB, 128), np.uint8)
    ohT = np.zeros((NCORES, 128, CB, 128), np.uint8)

    starts = np.zeros(NCORES * NBLK * 2 + 1, np.int64)
    starts[1:] = np.cumsum(counts.reshape(-1))

    # cb offset of each (block, half) segment, same for every core
    seg_off = {}
    cb0 = 0
    for g0 in range(0, NBLK, GBLK):
        blocks = list(range(g0, min(NBLK, g0 + GBLK)))
        off = cb0
        for b in blocks:
            seg_off[(b, 0)] = off
            off += tlo[b]
        for b in blocks:
            seg_off[(b, 1)] = off
            off += thi[b]
        cb0 = off
    assert cb0 == CB

    for c in range(NCORES):
        for b in range(NBLK):
            for hf in range(2):
                gi = (c * NBLK + b) * 2 + hf
                s, e = starts[gi], starts[gi + 1]
                n = int(e - s)
                if n == 0:
                    continue
                off = seg_off[(b, hf)]
                k = np.arange(n) + off * 128
                fi = (src[s:e] - hf * HALF).astype(np.int16)
                feat_idx[c, k] = fi
                p = k % 128
                t = k // 128
                r = (dst[s:e] - (c * SHARD + b * 128)).astype(np.int64)
                oh[c, p, t, r] = 1
                ohT[c, r, t, p] = 1

    oh8 = oh.astype(ml_dtypes.float8_e4m3fn).reshape(NCORES, 128, CB * 128)
    ohT8 = ohT.astype(ml_dtypes.float8_e4m3fn).reshape(NCORES, 128, CB * 128)

    in_maps = []
    for c in range(NCORES):
        xs = np.zeros((PAD_SHARD, D), np.float32)
        xs[0:SHARD] = x[c * SHARD : (c + 1) * SHARD]
        in_maps.append(
            {
                "x_shard": xs,
                "wextq": wextq,
                "c2q": c2b,
                "ident": ident,
                "qmat": qmat,
                "feat_idx": _wrap_idx(feat_idx[c]),
                "oh_d": np.ascontiguousarray(oh8[c]),
                "ohT_d": np.ascontiguousarray(ohT8[c]),
            }
        )
    return tlo, thi, na1, in_maps


_PROGRAM_CACHE = {}


def kernel(x, edge_index, edge_attr, h, batch, ln_gamma, ln_beta, W, att_src,
           att_dst, bias):
    x = np.asarray(x, dtype=np.float32)
    edge_index = np.asarray(edge_index)
    h = np.asarray(h)
    ln_gamma = np.asarray(ln_gamma, dtype=np.float32)
    ln_beta = np.asarray(ln_beta, dtype=np.float32)
    W = np.asarray(W, dtype=np.float32)
    att_src = np.asarray(att_src, dtype=np.float32)
    att_dst = np.asarray(att_dst, dtype=np.float32)
    bias = np.asarray(bias, dtype=np.float32)

    tlo, thi, na1, in_maps = _host_prep(
        x, edge_index, ln_gamma, ln_beta, W, att_src, att_dst, bias
    )
    key = (tlo, thi, round(na1, 6))
    if key not in _PROGRAM_CACHE:
        _PROGRAM_CACHE[key] = _build_program(tlo, thi, na1)
    nc = _PROGRAM_CACHE[key]

    res = run_bass_kernel_spmd(nc, in_maps, core_ids=list(range(NCORES)))
    out = np.concatenate([res.results[c]["out_shard"] for c in range(NCORES)], axis=0)
    return out, h
